# revision 2
# baseline (speedup 1.0000x reference)
"""Trainium2 Bass kernel for a Griffin-style ChimeraBlock:
   pre-norm RG-LRU recurrence branch + pre-norm SwiGLU FFN, B=2, T=2048,
   D=H=2048, FFN=5632, fp32 I/O.

Parallelization over 8 NeuronCores (tensor-parallel):
  - recurrence hidden dim H sharded 8x (256 per core); scan is elementwise
    per channel -> shards cleanly (native DVE tensor_tensor_scan op).
  - norm1 stats computed LOCALLY per core from the replicated bf16 x
    (square + ones-matmul partition reduction) -> no collective on the
    critical path into the recurrence.
  - AllGather of hs (bf16, raw) -> rec_out computed column-parallel with
    the rmsnorm scale applied post-matmul (per-column), gated on a tiny
    AllReduce of partial sum-of-squares that overlaps the matmuls.
  - xnew = x + rec_out AllGathered raw (bf16); norm2's scale is likewise
    applied post-matmul inside the FFN, so the AllReduce for its stats
    stays off the matmul critical path.
  - FFN hidden sharded 8x (704 -> padded 768 per core); down-proj partials
    ReduceScattered per 512-column chunk; each core emits its D-shard.
All weights are prefetched into SBUF at kernel start. Matmuls run in bf16
(fp32 accumulation in PSUM); gate/scan state in fp32; residual adds in
fp32. Host side only reshapes/transposes/casts/shards and folds the
(all-ones) rmsnorm gain vectors into adjacent weight matrices.
"""

import sys

sys.path.insert(0, "/opt/trn_rl_repo")

import numpy as np
import ml_dtypes

import concourse.bass as bass
import concourse.mybir as mybir
import concourse.tile as tile
from concourse import bacc
from concourse.bass_utils import run_bass_kernel_spmd

BF16 = mybir.dt.bfloat16
F32 = mybir.dt.float32
AF = mybir.ActivationFunctionType
OP = mybir.AluOpType

B, T, D = 2, 2048, 2048
H, FFN = 2048, 5632
NC = 8
HS = H // NC          # 256 hidden shard
DS = D // NC          # 256 d-model shard (output sharding)
FS = FFN // NC        # 704 ffn shard
FSP = 768             # ffn shard padded to a multiple of 128 (pad weights = 0)
BT = B * T            # 4096
CH = 512              # time-chunk (columns)
NCH = BT // CH        # 8 chunks
CPB = T // CH         # 4 chunks per batch element (scan resets at b boundary)
KD = D // 128         # 16 k-tiles when contracting over D
KH = H // 128         # 16 k-tiles when contracting over H
KF = FSP // 128       # 6 k-tiles when contracting over ffn shard
EPS = 1e-6
CCONST = 8.0

NP_BF16 = ml_dtypes.bfloat16


def _r128(ap):
    # [R, N] dram view -> [128, R//128, N] (partition, k-tile, col)
    return ap.rearrange("(k p) n -> p k n", p=128)


def build_nc():
    nc = bacc.Bacc("TRN2", target_bir_lowering=False, debug=False, num_devices=NC)
    rg = [list(range(NC))]

    # ---------------- kernel I/O (per core) ----------------
    xt = nc.dram_tensor("xt", [D, BT], BF16, kind="ExternalInput")      # x^T replicated
    xf32 = nc.dram_tensor("xf32", [DS, BT], F32, kind="ExternalInput")  # f32 x^T d-shard
    w3 = nc.dram_tensor("w3", [D, 3 * HS], BF16, kind="ExternalInput")  # in|ig|rg lhsT shard
    wro = nc.dram_tensor("wro", [H, DS], BF16, kind="ExternalInput")    # rec_out lhsT d-shard
    wg = nc.dram_tensor("wg", [D, FSP], BF16, kind="ExternalInput")
    wu = nc.dram_tensor("wu", [D, FSP], BF16, kind="ExternalInput")
    wd = nc.dram_tensor("wd", [FSP, D], BF16, kind="ExternalInput")
    # cols: 0 = rec_lambda, 1 = ig bias, 2 = rg bias, 3 = h0
    smalls = nc.dram_tensor("smalls", [HS, 4], F32, kind="ExternalInput")
    y = nc.dram_tensor("y", [DS, BT], F32, kind="ExternalOutput")

    with tile.TileContext(nc) as tc:
        with (
            tc.tile_pool(name="sb", bufs=2) as sb,
            tc.tile_pool(name="ps", bufs=2, space="PSUM") as ps,
            tc.tile_pool(name="dr", bufs=1, space="DRAM") as dr,
        ):
            build_body(nc, tc, sb, ps, dr, rg,
                       xt, xf32, w3, wro, wg, wu, wd, smalls, y)
    nc.compile()
    return nc


def build_body(nc, tc, sb, ps, dr, rg, xt, xf32, w3, wro, wg, wu, wd, smalls, y):
    AG = "AllGather"
    AR = "AllReduce"
    RS = "ReduceScatter"

    # ---------------- internal DRAM ----------------
    ar2_in = dr.tile([1, BT], F32, name="ar2_in")
    ar2_out = dr.tile([1, BT], F32, name="ar2_out", addr_space="Shared")
    ar3_in = dr.tile([1, BT], F32, name="ar3_in")
    ar3_out = dr.tile([1, BT], F32, name="ar3_out", addr_space="Shared")
    xnew_dram = dr.tile([DS, BT], BF16, name="xnew_dram")
    agin_hs = [dr.tile([HS, 2 * CH], BF16, name=f"agin_hs{j}") for j in range(4)]
    agout_hs = [dr.tile([H, 2 * CH], BF16, name=f"agout_hs{j}", addr_space="Shared")
                for j in range(4)]
    agin_h2 = [dr.tile([DS, 2 * CH], BF16, name=f"agin_h2{j}") for j in range(4)]
    agout_h2 = [dr.tile([D, 2 * CH], BF16, name=f"agout_h2{j}", addr_space="Shared")
                for j in range(4)]
    ffn_part = [dr.tile([D, CH], BF16, name=f"ffn_part{c}") for c in range(NCH)]
    ffn_red = [dr.tile([DS, CH], BF16, name=f"ffn_red{c}") for c in range(NCH)]

    dma = nc.sync.dma_start

    # ---------------- constants / small tensors ----------------
    ones_bf = sb.tile([128, 1], BF16, name="ones_bf", tag="ones", bufs=1)
    nc.vector.memset(ones_bf[:], 1.0)

    def const_tile(val, cname):
        t = sb.tile([128, 1], F32, name=cname, tag=cname, bufs=1)
        nc.vector.memset(t[:], val)
        return t

    c_ln8 = const_tile(1e-8, "c_ln8")         # Ln bias
    c_eps = const_tile(EPS, "c_eps")          # rmsnorm eps
    c_1eps = const_tile(1.0 + EPS, "c_1eps")  # 1 + eps for sqrt(1 - a^2 + eps)

    smalls_sb = sb.tile([128, 2, 4], F32, name="smalls_sb", tag="smalls", bufs=1)
    dma(out=smalls_sb[:], in_=smalls[:].rearrange("(a p) c -> p a c", p=128))
    sig_l = sb.tile([128, 2], F32, name="sig_l", tag="sig_l", bufs=1)
    nc.scalar.activation(sig_l[:], smalls_sb[:, :, 0], AF.Sigmoid)
    c8_sb = sb.tile([128, 2], F32, name="c8_sb", tag="c8", bufs=1)
    # log(sigmoid(lambda) + 1e-8)
    nc.scalar.activation(c8_sb[:], sig_l[:], AF.Ln, bias=c_ln8[:])
    # * C (in place via Copy with scale)
    nc.scalar.activation(c8_sb[:], c8_sb[:], AF.Copy, bias=0.0, scale=CCONST)

    # ---------------- weights: prefetch everything up front ----------------
    # wd shares w3's slot (same byte size); its DMA is emitted after P2 so
    # the slot is free by the time the down-proj needs it.
    w3_sb = sb.tile([128, KD, 3 * HS], BF16, name="w3_sb", tag="wbig", bufs=1)
    dma(out=w3_sb[:], in_=_r128(w3[:]))
    wro_sb = sb.tile([128, KH, DS], BF16, name="wro_sb", tag="wro", bufs=1)
    dma(out=wro_sb[:], in_=_r128(wro[:]))
    wg_sb = sb.tile([128, KD, FSP], BF16, name="wg_sb", tag="wg", bufs=1)
    dma(out=wg_sb[:], in_=_r128(wg[:]))
    wu_sb = sb.tile([128, KD, FSP], BF16, name="wu_sb", tag="wu", bufs=1)
    dma(out=wu_sb[:], in_=_r128(wu[:]))

    # ---------------- phase 2: local norm1 + in-proj + gates + scan ----------------
    hst_prev = None
    for c in range(NCH):
        cs = slice(c * CH, (c + 1) * CH)
        j, jj = c // 2, c % 2

        xc = sb.tile([128, KD, CH], BF16, name=f"xc{c}", tag="stream", bufs=2)
        dma(out=xc[:], in_=_r128(xt[:])[:, :, cs])

        # local sum-of-squares over all of D (x is replicated in bf16)
        psq1 = ps.tile([1, CH], F32, name=f"psq1_{c}", tag="psq", bufs=2)
        for q in range(KD // 2):
            xsqq = sb.tile([128, 2, CH], BF16, name=f"xsqq{c}_{q}", tag="sq3d",
                           bufs=3)
            nc.vector.tensor_tensor(xsqq[:], xc[:, 2 * q:2 * q + 2, :],
                                    xc[:, 2 * q:2 * q + 2, :], op=OP.mult)
            nc.tensor.matmul(psq1[:], ones_bf[:], xsqq[:, 0, :],
                             start=(q == 0), stop=False)
            nc.tensor.matmul(psq1[:], ones_bf[:], xsqq[:, 1, :],
                             start=False, stop=(q == KD // 2 - 1))
        arc = sb.tile([1, CH], F32, name=f"arc1_{c}", tag="row1", bufs=4)
        nc.scalar.activation(arc[:], psq1[:], AF.Sqrt, bias=c_eps[:1, :],
                             scale=1.0 / D)
        nc.vector.reciprocal(arc[:], arc[:])
        invc = sb.tile([128, CH], F32, name=f"invc1_{c}", tag="invcf", bufs=2)
        nc.gpsimd.partition_broadcast(invc[:], arc[:])

        zt = {}
        for m in range(2):
            for p_i in range(3):  # 0: x_proj, 1: input gate, 2: recurrence gate
                pst = ps.tile([128, CH], F32, name=f"pp{c}_{p_i}_{m}", tag="mm",
                              bufs=6)
                for k in range(KD):
                    nc.tensor.matmul(
                        pst[:],
                        w3_sb[:, k, p_i * HS + m * 128: p_i * HS + (m + 1) * 128],
                        xc[:, k, :],
                        start=(k == 0), stop=(k == KD - 1),
                    )
                z = sb.tile([128, CH], BF16, name=f"z{c}_{p_i}_{m}", tag="z",
                            bufs=6)
                nc.vector.tensor_tensor(z[:], pst[:], invc[:], op=OP.mult)
                zt[(p_i, m)] = z

        hst = sb.tile([128, 2, CH], BF16, name=f"hst{c}", tag="hs", bufs=3)
        for m in range(2):
            zx, zi, zr = zt[(0, m)], zt[(1, m)], zt[(2, m)]
            it = sb.tile([128, CH], BF16, name=f"it{c}_{m}", tag="it", bufs=2)
            nc.scalar.activation(it[:], zi[:], AF.Sigmoid,
                                 bias=smalls_sb[:, m, 1:2])
            rt = sb.tile([128, CH], F32, name=f"rt{c}_{m}", tag="rtna", bufs=4)
            nc.scalar.activation(rt[:], zr[:], AF.Sigmoid,
                                 bias=smalls_sb[:, m, 2:3])
            # la = r * (C * log_a)  (in place over rt)
            nc.vector.tensor_scalar_mul(rt[:], rt[:], c8_sb[:, m:m + 1])
            at = sb.tile([128, CH], F32, name=f"at{c}_{m}", tag="at", bufs=2)
            nc.scalar.activation(at[:], rt[:], AF.Exp)
            # na = -a^2 ; then sqrt(1 + eps - a^2)
            na = sb.tile([128, CH], F32, name=f"na{c}_{m}", tag="rtna", bufs=4)
            nc.vector.scalar_tensor_tensor(na[:], at[:], -1.0, at[:],
                                           op0=OP.mult, op1=OP.mult)
            nc.scalar.activation(na[:], na[:], AF.Sqrt, bias=c_1eps[:])
            # g = sq * (i * x_proj)   (build in place over zx)
            nc.vector.tensor_tensor(zx[:], it[:], zx[:], op=OP.mult)
            nc.vector.tensor_tensor(zx[:], na[:], zx[:], op=OP.mult)
            if c % CPB == 0:
                init = smalls_sb[:, m, 3:4]
            else:
                init = hst_prev[:, m, CH - 1:CH]
            nc.vector.tensor_tensor_scan(hst[:, m, :], at[:], zx[:], init,
                                         op0=OP.mult, op1=OP.add)
        hst_prev = hst

        # partial sumsq of hs over the h-shard
        hsq = sb.tile([128, 2, CH], BF16, name=f"hsq{c}", tag="sq3d", bufs=3)
        nc.vector.tensor_tensor(hsq[:], hst[:], hst[:], op=OP.mult)
        psq2 = ps.tile([1, CH], F32, name=f"psq2_{c}", tag="psq", bufs=2)
        nc.tensor.matmul(psq2[:], ones_bf[:], hsq[:, 0, :], start=True, stop=False)
        nc.tensor.matmul(psq2[:], ones_bf[:], hsq[:, 1, :], start=False, stop=True)
        sqs2 = sb.tile([1, CH], F32, name=f"sqs2_{c}", tag="row1", bufs=4)
        nc.scalar.copy(sqs2[:], psq2[:])
        dma(out=ar2_in[0:1, cs], in_=sqs2[:])
        dma(out=_r128(agin_hs[j][:])[:, :, jj * CH:(jj + 1) * CH], in_=hst[:])
        if jj == 1:
            nc.gpsimd.collective_compute(AG, OP.bypass, replica_groups=rg,
                                         ins=[agin_hs[j][:]], outs=[agout_hs[j][:]])

    nc.gpsimd.collective_compute(AR, OP.add, replica_groups=rg,
                                 ins=[ar2_in[:]], outs=[ar2_out[:]])

    # down-proj weights go into w3's (now free) slot; DMA overlaps P4.
    wd_sb = sb.tile([128, KF, D], BF16, name="wd_sb", tag="wbig", bufs=1)
    dma(out=wd_sb[:], in_=_r128(wd[:]))

    # ---------------- phase 4: rec_out (d-shard) + residual + norm2 stats ----------------
    for c in range(NCH):
        cs = slice(c * CH, (c + 1) * CH)
        j, jj = c // 2, c % 2
        arc2 = sb.tile([1, CH], F32, name=f"arc2_{c}", tag="row1", bufs=4)
        dma(out=arc2[:], in_=ar2_out[0:1, cs])
        nc.scalar.activation(arc2[:], arc2[:], AF.Sqrt, bias=c_eps[:1, :],
                             scale=1.0 / H)
        nc.vector.reciprocal(arc2[:], arc2[:])
        invc2 = sb.tile([128, CH], F32, name=f"invc2_{c}", tag="invcf", bufs=2)
        nc.gpsimd.partition_broadcast(invc2[:], arc2[:])

        # rec_out matmuls on the raw (un-normalized) gathered hs; the rmsnorm
        # scale is applied post-matmul (it is per-column). hs streamed in two
        # half-H pieces to halve the SBUF stream footprint.
        psts = [ps.tile([128, CH], F32, name=f"pro{c}_{m}", tag="mm", bufs=6)
                for m in range(2)]
        for h in range(2):
            hstm = sb.tile([128, KH // 2, CH], BF16, name=f"hstm{c}_{h}",
                           tag="hstm", bufs=2)
            dma(out=hstm[:],
                in_=_r128(agout_hs[j][:])[:, h * (KH // 2):(h + 1) * (KH // 2),
                                          jj * CH:(jj + 1) * CH])
            for m in range(2):
                for k in range(KH // 2):
                    nc.tensor.matmul(psts[m][:],
                                     wro_sb[:, h * (KH // 2) + k,
                                            m * 128:(m + 1) * 128],
                                     hstm[:, k, :],
                                     start=(h == 0 and k == 0),
                                     stop=(h == 1 and k == KH // 2 - 1))
        xnt = sb.tile([128, 2, CH], BF16, name=f"xnt{c}", tag="xnt", bufs=2)
        for m in range(2):
            # scale by inv rms (in place in PSUM), add residual, downcast
            nc.vector.tensor_tensor(psts[m][:], psts[m][:], invc2[:], op=OP.mult)
            xft = sb.tile([128, CH], F32, name=f"xft{c}_{m}", tag="xf", bufs=2)
            dma(out=xft[:], in_=_r128(xf32[:])[:, m, cs])
            nc.vector.tensor_tensor(xnt[:, m, :], psts[m][:], xft[:], op=OP.add)
        xnq = sb.tile([128, 2, CH], BF16, name=f"xnq{c}", tag="sq3d", bufs=3)
        nc.vector.tensor_tensor(xnq[:], xnt[:], xnt[:], op=OP.mult)
        psq3 = ps.tile([1, CH], F32, name=f"psq3_{c}", tag="psq", bufs=2)
        nc.tensor.matmul(psq3[:], ones_bf[:], xnq[:, 0, :], start=True, stop=False)
        nc.tensor.matmul(psq3[:], ones_bf[:], xnq[:, 1, :], start=False, stop=True)
        sqs3 = sb.tile([1, CH], F32, name=f"sqs3_{c}", tag="row1", bufs=4)
        nc.scalar.copy(sqs3[:], psq3[:])
        dma(out=ar3_in[0:1, cs], in_=sqs3[:])
        # xnew (raw, bf16) goes out for AllGather and is kept for the final
        # residual; norm2's scale is applied post-matmul in the FFN.
        dma(out=_r128(agin_h2[j][:])[:, :, jj * CH:(jj + 1) * CH], in_=xnt[:])
        dma(out=_r128(xnew_dram[:])[:, :, cs], in_=xnt[:])
        if jj == 1:
            nc.gpsimd.collective_compute(AG, OP.bypass, replica_groups=rg,
                                         ins=[agin_h2[j][:]], outs=[agout_h2[j][:]])

    nc.gpsimd.collective_compute(AR, OP.add, replica_groups=rg,
                                 ins=[ar3_in[:]], outs=[ar3_out[:]])

    # ---------------- phase 6: FFN on raw gathered xnew ----------------
    for c in range(NCH):
        j, jj = c // 2, c % 2
        arc3 = sb.tile([1, CH], F32, name=f"arc3_{c}", tag="row1", bufs=4)
        dma(out=arc3[:], in_=ar3_out[0:1, c * CH:(c + 1) * CH])
        nc.scalar.activation(arc3[:], arc3[:], AF.Sqrt, bias=c_eps[:1, :],
                             scale=1.0 / D)
        nc.vector.reciprocal(arc3[:], arc3[:])
        invc3 = sb.tile([128, CH], F32, name=f"invc3_{c}", tag="invcf", bufs=2)
        nc.gpsimd.partition_broadcast(invc3[:], arc3[:])

        h2s = sb.tile([128, KD, CH], BF16, name=f"h2s{c}", tag="stream", bufs=2)
        dma(out=h2s[:], in_=_r128(agout_h2[j][:])[:, :, jj * CH:(jj + 1) * CH])
        gu = sb.tile([128, KF, CH], BF16, name=f"gu{c}", tag="gu", bufs=1)
        for m in range(KF):
            psg = ps.tile([128, CH], F32, name=f"pg{c}_{m}", tag="mm", bufs=6)
            for k in range(KD):
                nc.tensor.matmul(psg[:], wg_sb[:, k, m * 128:(m + 1) * 128],
                                 h2s[:, k, :],
                                 start=(k == 0), stop=(k == KD - 1))
            psu = ps.tile([128, CH], F32, name=f"pu{c}_{m}", tag="mm", bufs=6)
            for k in range(KD):
                nc.tensor.matmul(psu[:], wu_sb[:, k, m * 128:(m + 1) * 128],
                                 h2s[:, k, :],
                                 start=(k == 0), stop=(k == KD - 1))
            # normalize the gate pre-activation per column, then silu
            t1 = sb.tile([128, CH], BF16, name=f"t1_{c}_{m}", tag="bf1", bufs=6)
            nc.vector.tensor_tensor(t1[:], psg[:], invc3[:], op=OP.mult)
            gs = sb.tile([128, CH], BF16, name=f"gs{c}_{m}", tag="bf1", bufs=6)
            nc.scalar.activation(gs[:], t1[:], AF.Silu)
            v = sb.tile([128, CH], BF16, name=f"v{c}_{m}", tag="bf1", bufs=6)
            nc.vector.tensor_tensor(v[:], psu[:], invc3[:], op=OP.mult)
            nc.vector.tensor_tensor(gu[:, m, :], gs[:], v[:], op=OP.mult)
        for m in range(KD):
            psd = ps.tile([128, CH], F32, name=f"pd{c}_{m}", tag="mm", bufs=6)
            for k in range(KF):
                nc.tensor.matmul(psd[:], wd_sb[:, k, m * 128:(m + 1) * 128],
                                 gu[:, k, :],
                                 start=(k == 0), stop=(k == KF - 1))
            dst = sb.tile([128, CH], BF16, name=f"dst{c}_{m}", tag="bf1", bufs=6)
            nc.vector.tensor_copy(dst[:], psd[:])
            dma(out=ffn_part[c][m * 128:(m + 1) * 128, :], in_=dst[:])
        nc.gpsimd.collective_compute(RS, OP.add, replica_groups=rg,
                                     ins=[ffn_part[c][:]], outs=[ffn_red[c][:]])

    # ---------------- phase 7: final residual ----------------
    for c in range(NCH):
        cs = slice(c * CH, (c + 1) * CH)
        for m in range(2):
            frt = sb.tile([128, CH], BF16, name=f"frt{c}_{m}", tag="frt", bufs=2)
            dma(out=frt[:], in_=_r128(ffn_red[c][:])[:, m, :])
            xb = sb.tile([128, CH], BF16, name=f"xb{c}_{m}", tag="xb", bufs=2)
            dma(out=xb[:], in_=_r128(xnew_dram[:])[:, m, cs])
            yt = sb.tile([128, CH], F32, name=f"yt{c}_{m}", tag="yt", bufs=2)
            nc.vector.tensor_tensor(yt[:], xb[:], frt[:], op=OP.add)
            dma(out=_r128(y[:])[:, m, cs], in_=yt[:])


_CACHE = {}


def _prep_inputs(inputs):
    f = np.float32
    x = np.asarray(inputs["x"], f)                       # [B, T, D]
    norm1_w = np.asarray(inputs["norm1_w"], f)
    rec_in_w = np.asarray(inputs["rec_in_w"], f)         # [H, D]
    rec_ig_w = np.asarray(inputs["rec_ig_w"], f)
    rec_ig_b = np.asarray(inputs["rec_ig_b"], f)
    rec_rg_w = np.asarray(inputs["rec_rg_w"], f)
    rec_rg_b = np.asarray(inputs["rec_rg_b"], f)
    rec_lambda = np.asarray(inputs["rec_lambda"], f)
    rec_out_w = np.asarray(inputs["rec_out_w"], f)       # [D, H]
    rec_h0 = np.asarray(inputs["rec_h0"], f)             # [1, 1, H]
    rec_norm_w = np.asarray(inputs["rec_norm_w"], f)
    norm2_w = np.asarray(inputs["norm2_w"], f)
    ffn_gate_w = np.asarray(inputs["ffn_gate_w"], f)     # [FFN, D]
    ffn_up_w = np.asarray(inputs["ffn_up_w"], f)
    ffn_down_w = np.asarray(inputs["ffn_down_w"], f)     # [D, FFN]

    xt_full = np.ascontiguousarray(
        x.reshape(BT, D).T.astype(NP_BF16))              # [D, BT]
    xt_f32 = np.ascontiguousarray(x.reshape(BT, D).T)    # [D, BT] f32

    # fold norm gains into adjacent weights; transpose into lhsT layouts
    w_in_t = (rec_in_w * norm1_w[None, :]).T             # [D, H]
    w_ig_t = (rec_ig_w * norm1_w[None, :]).T
    w_rg_t = (rec_rg_w * norm1_w[None, :]).T
    w_ro_t = (rec_out_w * rec_norm_w[None, :]).T         # [H, D]
    w_g_t = (ffn_gate_w * norm2_w[None, :]).T            # [D, FFN]
    w_u_t = (ffn_up_w * norm2_w[None, :]).T
    w_d_t = ffn_down_w.T                                 # [FFN, D]

    in_maps = []
    for r in range(NC):
        hsl = slice(r * HS, (r + 1) * HS)
        dsl = slice(r * DS, (r + 1) * DS)
        fsl = slice(r * FS, (r + 1) * FS)
        w3_r = np.concatenate(
            [w_in_t[:, hsl], w_ig_t[:, hsl], w_rg_t[:, hsl]], axis=1)
        wg_r = np.zeros((D, FSP), f)
        wg_r[:, :FS] = w_g_t[:, fsl]
        wu_r = np.zeros((D, FSP), f)
        wu_r[:, :FS] = w_u_t[:, fsl]
        wd_r = np.zeros((FSP, D), f)
        wd_r[:FS, :] = w_d_t[fsl, :]
        smalls_r = np.stack(
            [rec_lambda[hsl], rec_ig_b[hsl], rec_rg_b[hsl],
             np.broadcast_to(rec_h0[0, 0], (H,))[hsl]], axis=1)
        in_maps.append({
            "xt": xt_full,
            "xf32": np.ascontiguousarray(xt_f32[dsl, :]),
            "w3": np.ascontiguousarray(w3_r.astype(NP_BF16)),
            "wro": np.ascontiguousarray(w_ro_t[:, dsl].astype(NP_BF16)),
            "wg": np.ascontiguousarray(wg_r.astype(NP_BF16)),
            "wu": np.ascontiguousarray(wu_r.astype(NP_BF16)),
            "wd": np.ascontiguousarray(wd_r.astype(NP_BF16)),
            "smalls": np.ascontiguousarray(smalls_r.astype(f)),
        })
    return in_maps


def run_on_device(inputs, trace=False, tmpdir=None):
    if "nc" not in _CACHE:
        _CACHE["nc"] = build_nc()
    nc = _CACHE["nc"]
    in_maps = _prep_inputs(inputs)
    res = run_bass_kernel_spmd(nc, in_maps, list(range(NC)),
                               trace=trace, tmpdir=tmpdir)
    shards = [np.asarray(res.results[r]["y"]) for r in range(NC)]
    yt = np.concatenate(shards, axis=0)                  # [D, BT]
    out = np.ascontiguousarray(yt.T).reshape(B, T, D).astype(np.float32)
    return out, res


def kernel(**inputs):
    out, _ = run_on_device(inputs, trace=False)
    return out
